# revision 6
# baseline (speedup 1.0000x reference)
"""Multi-head latent attention (MLA) TRN2 kernel.

Sharding: batch(2) x query-sequence(4) over 8 cores. Each core:
  - computes the full KV path for its batch (kv_a, rmsnorm, kv_b, rope)
  - computes the Q path for its 512-token query chunk
  - full attention for its 512 queries x 2048 keys x 16 heads
  - o_proj for its chunk -> output slice [512, 2048]
Host assembles the 8 slices into [B, T, HID]. No collectives.

All matmuls run in float32r (fp32 with 11-bit mantissa, 1 cycle/row on the
PE when N>=256 -- same throughput as bf16 at ~2^-12 relative precision).
Activations are kept feature-major ([feature, token]) so weight matrices act
as lhsT directly as stored; attention computes scores transposed
(s^T[k,q] = k^T q) so softmax needs no transposes: exp on ACT, the
denominator via an all-ones lhsT matmul (broadcast into all 128 partitions),
and P@V consumes the transposed probabilities directly.
"""

import math

import numpy as np

B, T, HID = 2, 2048, 2048
NH, NKV = 16, 8
NOPE, ROPE = 128, 64
HD = NOPE + ROPE  # 192
VD = 128
KV_RANK, Q_RANK = 512, 1536
EPS = 1e-6
THETA = 10000.0
NCORES = 8
TQ = B * T // NCORES  # 512 query tokens per core
P = 128
SCALE = 1.0 / math.sqrt(HD)

# Rope rows are stored "paired": each head's rotated rope halves (32+32 rows)
# are stacked into one contiguous 64-row slot, two heads per 128-partition
# tile, at base partition 64*(kvh%2) so score-matmul lhsT(k)/rhs(q) base
# partitions match (PE only allows bases {0, 32, 64}).

_CACHE = {}


def _round_f32r(a):
    a = np.ascontiguousarray(np.asarray(a, dtype=np.float32))
    u = a.view(np.uint32)
    low = u & np.uint32(0xFFF)
    rounded = u & np.uint32(0xFFFFF000)
    lsb = (u >> np.uint32(12)) & np.uint32(1)
    round_up = (low > 0x800) | ((low == 0x800) & (lsb == 1))
    return (rounded + (round_up.astype(np.uint32) << np.uint32(12))).view(np.float32)


def _build_nc():
    import concourse.bass as bass  # noqa: F401
    import concourse.mybir as mybir
    from concourse import bacc
    from concourse.tile import TileContext

    F32 = mybir.dt.float32
    F32R = mybir.dt.float32r
    AF = mybir.ActivationFunctionType
    ALU = mybir.AluOpType

    nc = bacc.Bacc(None, target_bir_lowering=False)

    xT = nc.dram_tensor("xT", [HID, T], F32R, kind="ExternalInput")
    xq = nc.dram_tensor("xq", [HID, TQ], F32R, kind="ExternalInput")
    qa_w = nc.dram_tensor("qa_w", [HID, Q_RANK], F32R, kind="ExternalInput")
    qa_ln = nc.dram_tensor("qa_ln", [P, Q_RANK // P], F32R, kind="ExternalInput")
    qb_w = nc.dram_tensor("qb_w", [Q_RANK, NH * HD], F32R, kind="ExternalInput")
    kva_w = nc.dram_tensor("kva_w", [HID, KV_RANK + NKV * ROPE], F32R, kind="ExternalInput")
    kva_ln = nc.dram_tensor("kva_ln", [P, KV_RANK // P], F32R, kind="ExternalInput")
    kvb_w = nc.dram_tensor("kvb_w", [KV_RANK, NKV * (NOPE + VD)], F32R, kind="ExternalInput")
    o_w = nc.dram_tensor("o_w", [NH * VD, HID], F32R, kind="ExternalInput")
    cosq = nc.dram_tensor("cosq", [P, TQ], F32R, kind="ExternalInput")
    sinq = nc.dram_tensor("sinq", [P, TQ], F32R, kind="ExternalInput")
    cosk = nc.dram_tensor("cosk", [P, T], F32R, kind="ExternalInput")
    sink = nc.dram_tensor("sink", [P, T], F32R, kind="ExternalInput")
    ones_in = nc.dram_tensor("ones_in", [P, P], F32R, kind="ExternalInput")
    out = nc.dram_tensor("out", [TQ, HID], F32, kind="ExternalOutput")

    xT_t = xT.rearrange("(kt p) t -> p kt t", p=P)  # [128, 16, T]
    xq_t = xq.rearrange("(kt p) t -> p kt t", p=P)  # [128, 16, TQ]

    with TileContext(nc) as tc:
        with (
            tc.tile_pool(name="tables", bufs=1) as tbl,
            tc.tile_pool(name="dram", bufs=1, space="DRAM") as dpool,
        ):
            ones_sb = tbl.tile([P, P], F32R, name="ones_sb")
            nc.sync.dma_start(ones_sb[:], ones_in[:, :])
            lnq_sb = tbl.tile([P, Q_RANK // P], F32R, name="lnq_sb")
            nc.sync.dma_start(lnq_sb[:], qa_ln[:, :])
            lnkv_sb = tbl.tile([P, KV_RANK // P], F32R, name="lnkv_sb")
            nc.sync.dma_start(lnkv_sb[:], kva_ln[:, :])

            epskv_sb = tbl.tile([P, 1], F32, name="epskv_sb")
            nc.gpsimd.memset(epskv_sb[:], float(EPS * KV_RANK))
            epsq_sb = tbl.tile([P, 1], F32, name="epsq_sb")
            nc.gpsimd.memset(epsq_sb[:], float(EPS * Q_RANK))

            kpaird = dpool.tile([P, 4, T], F32R, name="kpaird")
            latNd = dpool.tile([P, 4, T], F32R, name="latNd")
            attnd = dpool.tile([P, NH, TQ], F32R, name="attnd")

            with tc.tile_pool(name="pA", bufs=1) as pA:  # noqa: F841

                # ---------------- P1a: kv_a latent + rmsnorm ----------------
                with (
                    tc.tile_pool(name="p1a", bufs=1) as p1,
                    tc.tile_pool(name="p1as", bufs=2) as p1s,
                    tc.tile_pool(name="p1aps", bufs=2, space="PSUM") as p1ps,
                    tc.tile_pool(name="p1aps1", bufs=1, space="PSUM") as p1ps1,
                ):
                    kv_latN = p1.tile([P, 4, T], F32R, name="kv_latN")
                    kvaw_sb = p1.tile([P, 16, KV_RANK], F32R, name="kvaw_sb")
                    nc.sync.dma_start(
                        kvaw_sb[:],
                        kva_w.rearrange("(kt p) c -> p kt c", p=P)[:, :, :KV_RANK],
                    )
                    rs_kv = p1.tile([P, 4, 512], F32, name="rs_kv")

                    for nch in range(4):
                        CW = 512
                        xch = p1s.tile([P, 16, CW], F32R, tag="xch")
                        nc.sync.dma_start(xch[:], xT_t[:, :, nch * CW : (nch + 1) * CW])
                        sumsq = p1ps1.tile([P, CW], F32, tag="sumsq")
                        for m in range(4):
                            ps = p1ps.tile([P, CW], F32, tag="kva_ps")
                            for k in range(16):
                                nc.tensor.matmul(
                                    ps[:],
                                    kvaw_sb[:, k, m * P : (m + 1) * P],
                                    xch[:, k, :],
                                    start=(k == 0),
                                    stop=(k == 15),
                                )
                            nc.vector.tensor_copy(
                                kv_latN[:, m, nch * CW : (nch + 1) * CW], ps[:]
                            )
                            sq = p1s.tile([P, CW], F32R, tag="sq")
                            nc.scalar.square(sq[:], ps[:])
                            nc.tensor.matmul(
                                sumsq[:], ones_sb[:], sq[:],
                                start=(m == 0), stop=(m == 3),
                            )
                        sqt = p1s.tile([P, CW], F32, tag="sqt")
                        nc.scalar.activation(
                            sqt[:], sumsq[:], AF.Sqrt, bias=epskv_sb[:]
                        )
                        nc.vector.reciprocal(rs_kv[:, nch, :], sqt[:])
                        for m in range(4):
                            nc.vector.scalar_tensor_tensor(
                                kv_latN[:, m, nch * CW : (nch + 1) * CW],
                                kv_latN[:, m, nch * CW : (nch + 1) * CW],
                                lnkv_sb[:, m : m + 1],
                                rs_kv[:, nch, :],
                                ALU.mult,
                                ALU.mult,
                            )
                            nc.sync.dma_start(
                                latNd[:, m, nch * CW : (nch + 1) * CW],
                                kv_latN[:, m, nch * CW : (nch + 1) * CW],
                            )

                # ---------------- P1b: kv_a rope + rotation -----------------
                with (
                    tc.tile_pool(name="p1b", bufs=1) as p1,
                    tc.tile_pool(name="p1bs", bufs=2) as p1s,
                    tc.tile_pool(name="p1bps", bufs=2, space="PSUM") as p1ps,
                ):
                    kvawr_sb = p1.tile([P, 16, NKV * ROPE], F32R, name="kvawr_sb")
                    nc.sync.dma_start(
                        kvawr_sb[:],
                        kva_w.rearrange("(kt p) c -> p kt c", p=P)[:, :, KV_RANK:],
                    )
                    raw1 = p1.tile([P, 2, T], F32R, name="kraw1")
                    raw2 = p1.tile([P, 2, T], F32R, name="kraw2")

                    NCH = 8
                    CW = T // NCH  # 256
                    for nch in range(NCH):
                        xch = p1s.tile([P, 16, CW], F32R, tag="xchb")
                        nc.sync.dma_start(xch[:], xT_t[:, :, nch * CW : (nch + 1) * CW])
                        for m in range(4):
                            ps = p1ps.tile([P, CW], F32, tag="kvab_ps")
                            for k in range(16):
                                nc.tensor.matmul(
                                    ps[:],
                                    kvawr_sb[:, k, m * P : (m + 1) * P],
                                    xch[:, k, :],
                                    start=(k == 0),
                                    stop=(k == 15),
                                )
                            if m < 2:
                                nc.scalar.copy(
                                    raw1[:, m, nch * CW : (nch + 1) * CW], ps[:]
                                )
                            else:
                                nc.scalar.copy(
                                    raw2[:, m - 2, nch * CW : (nch + 1) * CW], ps[:]
                                )

                    # k-rope rotation (block layout, batched DVE) -> HBM paired
                    cosk_sb = p1.tile([P, T], F32R, name="cosk_sb")
                    nc.sync.dma_start(cosk_sb[:], cosk[:, :])
                    sink_sb = p1.tile([P, T], F32R, name="sink_sb")
                    nc.sync.dma_start(sink_sb[:], sink[:, :])
                    for t in range(2):
                        tmp = p1s.tile([P, T], F32R, tag="rot_tmp")
                        rot = p1s.tile([P, T], F32R, tag="rot_out")
                        nc.vector.tensor_tensor(tmp[:], raw2[:, t, :], sink_sb[:], ALU.mult)
                        nc.vector.tensor_tensor(rot[:], raw1[:, t, :], cosk_sb[:], ALU.mult)
                        nc.vector.tensor_tensor(rot[:], rot[:], tmp[:], ALU.subtract)
                        tmp2 = p1s.tile([P, T], F32R, tag="rot_tmp")
                        rot2 = p1s.tile([P, T], F32R, tag="rot_out")
                        nc.vector.tensor_tensor(tmp2[:], raw1[:, t, :], sink_sb[:], ALU.mult)
                        nc.vector.tensor_tensor(rot2[:], raw2[:, t, :], cosk_sb[:], ALU.mult)
                        nc.vector.tensor_tensor(rot2[:], rot2[:], tmp2[:], ALU.add)
                        # scatter into paired HBM layout: head kvh=4t+i at
                        # tile kvh//2, base 64*(kvh%2), rot1 at +0, rot2 at +32
                        for i in range(4):
                            kvh = 4 * t + i
                            bb = 64 * (kvh % 2)
                            nc.sync.dma_start(
                                kpaird[bb : bb + 32, kvh // 2, :],
                                rot[i * 32 : (i + 1) * 32, :],
                            )
                            nc.sync.dma_start(
                                kpaird[bb + 32 : bb + 64, kvh // 2, :],
                                rot2[i * 32 : (i + 1) * 32, :],
                            )

                # ---------------- P2: q path --------------------------------
                with tc.tile_pool(name="pQ", bufs=1) as pQ:
                    q_nope = pQ.tile([P, NH, TQ], F32R, name="q_nope")
                    qpair = pQ.tile([P, 8, TQ], F32R, name="qpair")

                    with (
                        tc.tile_pool(name="p2", bufs=1) as p2,
                        tc.tile_pool(name="p2s", bufs=2) as p2s,
                        tc.tile_pool(name="p2ps", bufs=2, space="PSUM") as p2ps,
                        tc.tile_pool(name="p2ps1", bufs=1, space="PSUM") as p2ps1,
                    ):
                        q_lat = p2.tile([P, Q_RANK // P, TQ], F32R, name="q_lat")
                        xq_sb = p2.tile([P, 16, TQ], F32R, name="xq_sb")
                        nc.sync.dma_start(xq_sb[:], xq_t[:, :, :])

                        # q_a + rmsnorm
                        sumsq = p2ps1.tile([P, TQ], F32, tag="qsumsq")
                        for m in range(12):
                            wt = p2s.tile([P, 16, P], F32R, tag="qa_wt")
                            nc.sync.dma_start(
                                wt[:],
                                qa_w.rearrange("(kt p) c -> p kt c", p=P)[
                                    :, :, m * P : (m + 1) * P
                                ],
                            )
                            ps = p2ps.tile([P, TQ], F32, tag="qa_ps")
                            for k in range(16):
                                nc.tensor.matmul(
                                    ps[:],
                                    wt[:, k, :],
                                    xq_sb[:, k, :],
                                    start=(k == 0),
                                    stop=(k == 15),
                                )
                            nc.vector.tensor_copy(q_lat[:, m, :], ps[:])
                            sq = p2s.tile([P, TQ], F32R, tag="qsq")
                            nc.scalar.square(sq[:], ps[:])
                            nc.tensor.matmul(
                                sumsq[:], ones_sb[:], sq[:],
                                start=(m == 0), stop=(m == 11),
                            )
                        sqt = p2s.tile([P, TQ], F32, tag="qsqt")
                        nc.scalar.activation(
                            sqt[:], sumsq[:], AF.Sqrt, bias=epsq_sb[:]
                        )
                        rs_q = p2.tile([P, TQ], F32, name="rs_q")
                        nc.vector.reciprocal(rs_q[:], sqt[:])
                        for m in range(Q_RANK // P):
                            nc.vector.scalar_tensor_tensor(
                                q_lat[:, m, :],
                                q_lat[:, m, :],
                                lnq_sb[:, m : m + 1],
                                rs_q[:],
                                ALU.mult,
                                ALU.mult,
                            )

                        # q_b
                        qraw1 = p2.tile([P, 4, TQ], F32R, name="qraw1")
                        qraw2 = p2.tile([P, 4, TQ], F32R, name="qraw2")
                        for m in range(24):
                            wt = p2s.tile([P, 12, P], F32R, tag="qb_wt")
                            nc.sync.dma_start(
                                wt[:],
                                qb_w.rearrange("(kt p) c -> p kt c", p=P)[
                                    :, :, m * P : (m + 1) * P
                                ],
                            )
                            ps = p2ps.tile([P, TQ], F32, tag="qb_ps")
                            for k in range(12):
                                nc.tensor.matmul(
                                    ps[:],
                                    wt[:, k, :],
                                    q_lat[:, k, :],
                                    start=(k == 0),
                                    stop=(k == 11),
                                )
                            if m < 16:
                                nc.scalar.copy(q_nope[:, m, :], ps[:])
                            elif m < 20:
                                nc.scalar.copy(qraw1[:, m - 16, :], ps[:])
                            else:
                                nc.scalar.copy(qraw2[:, m - 20, :], ps[:])

                        # q-rope rotation (batched over all 4 tiles)
                        cosq_sb = p2.tile([P, TQ], F32R, name="cosq_sb")
                        nc.sync.dma_start(cosq_sb[:], cosq[:, :])
                        sinq_sb = p2.tile([P, TQ], F32R, name="sinq_sb")
                        nc.sync.dma_start(sinq_sb[:], sinq[:, :])
                        cb = cosq_sb[:, None, :].to_broadcast((P, 4, TQ))
                        sb = sinq_sb[:, None, :].to_broadcast((P, 4, TQ))
                        qrot1 = p2.tile([P, 4, TQ], F32R, name="qrot1")
                        qrot2 = p2.tile([P, 4, TQ], F32R, name="qrot2")
                        tmp = p2.tile([P, 4, TQ], F32R, name="qrot_tmp1")
                        nc.vector.tensor_tensor(tmp[:], qraw2[:], sb, ALU.mult)
                        nc.vector.tensor_tensor(qrot1[:], qraw1[:], cb, ALU.mult)
                        nc.vector.tensor_tensor(qrot1[:], qrot1[:], tmp[:], ALU.subtract)
                        tmp2 = p2.tile([P, 4, TQ], F32R, name="qrot_tmp2")
                        nc.vector.tensor_tensor(tmp2[:], qraw1[:], sb, ALU.mult)
                        nc.vector.tensor_tensor(qrot2[:], qraw2[:], cb, ALU.mult)
                        nc.vector.tensor_tensor(qrot2[:], qrot2[:], tmp2[:], ALU.add)
                        # scatter rotated q-rope into paired layout: head h at
                        # tile 2*(h//4)+h%2, base 64*((h//2)%2)
                        for h in range(NH):
                            tq_ = 2 * (h // 4) + h % 2
                            bb = 64 * ((h // 2) % 2)
                            nc.sync.dma_start(
                                qpair[bb : bb + 32, tq_, :],
                                qrot1[(h % 4) * 32 : (h % 4) * 32 + 32, h // 4, :],
                            )
                            nc.sync.dma_start(
                                qpair[bb + 32 : bb + 64, tq_, :],
                                qrot2[(h % 4) * 32 : (h % 4) * 32 + 32, h // 4, :],
                            )

                    # ---------------- P3: attention -------------------------
                    with (
                        tc.tile_pool(name="p3", bufs=1) as p3,
                        tc.tile_pool(name="p3s", bufs=2) as p3s,
                        tc.tile_pool(name="p3p", bufs=3) as p3p,
                        tc.tile_pool(name="scps", bufs=2, space="PSUM") as scps,
                        tc.tile_pool(name="dnps", bufs=2, space="PSUM") as dnps,
                        tc.tile_pool(name="atps", bufs=2, space="PSUM") as atps,
                        tc.tile_pool(name="prps", bufs=1, space="PSUM") as prps,
                    ):
                        kv_latN = p3.tile([P, 4, T], F32R, name="kv_latN3")
                        nc.sync.dma_start(kv_latN[:], latNd[:, :, :])
                        for hp in range(4):  # kv-head pairs
                            kvh0 = 2 * hp
                            wn = p3s.tile([P, 4, 256], F32R, tag="wn")
                            nc.sync.dma_start(
                                wn[:],
                                kvb_w.rearrange("(kt p) c -> p kt c", p=P)[
                                    :, :, kvh0 * NOPE : (kvh0 + 2) * NOPE
                                ],
                            )
                            wv = p3s.tile([P, 4, 256], F32R, tag="wv")
                            nc.sync.dma_start(
                                wv[:],
                                kvb_w.rearrange("(kt p) c -> p kt c", p=P)[
                                    :, :, NKV * NOPE + kvh0 * VD : NKV * NOPE + (kvh0 + 2) * VD
                                ],
                            )
                            knp = p3s.tile([P, 2, T], F32R, tag="knp")
                            for h2 in range(2):
                                for nch in range(4):
                                    ps = prps.tile([P, 512], F32, tag="knp_ps")
                                    for k in range(4):
                                        nc.tensor.matmul(
                                            ps[:],
                                            wn[:, k, h2 * P : (h2 + 1) * P],
                                            kv_latN[:, k, nch * 512 : (nch + 1) * 512],
                                            start=(k == 0),
                                            stop=(k == 3),
                                        )
                                    nc.scalar.copy(
                                        knp[:, h2, nch * 512 : (nch + 1) * 512], ps[:]
                                    )
                            vp = p3s.tile([P, 16, 256], F32R, tag="vp")
                            for mt in range(16):
                                ps = prps.tile([P, 256], F32, tag="v_ps")
                                for k in range(4):
                                    nc.tensor.matmul(
                                        ps[:],
                                        kv_latN[:, k, mt * P : (mt + 1) * P],
                                        wv[:, k, :],
                                        start=(k == 0),
                                        stop=(k == 3),
                                    )
                                nc.scalar.copy(vp[:, mt, :], ps[:])
                            krp = p3s.tile([P, T], F32R, tag="krp")
                            nc.sync.dma_start(krp[:], kpaird[:, hp, :])

                            for j4 in range(4):
                                qh = 4 * hp + j4
                                kvh = qh // 2
                                h2 = kvh - kvh0
                                b = 64 * (kvh % 2)
                                tq_ = 2 * (qh // 4) + qh % 2
                                dn = dnps.tile([P, TQ], F32, tag="dn")
                                at = atps.tile([P, TQ], F32, tag="at")
                                for kt in range(16):
                                    sc = scps.tile([P, TQ], F32, tag="sc")
                                    nc.tensor.matmul(
                                        sc[:],
                                        knp[:, h2, kt * P : (kt + 1) * P],
                                        q_nope[:, qh, :],
                                        start=True,
                                        stop=False,
                                    )
                                    nc.tensor.matmul(
                                        sc[:],
                                        krp[b : b + 64, kt * P : (kt + 1) * P],
                                        qpair[b : b + 64, tq_, :],
                                        start=False,
                                        stop=True,
                                    )
                                    pt = p3p.tile([P, TQ], F32R, tag="probsT")
                                    nc.scalar.activation(
                                        pt[:], sc[:], AF.Exp, scale=float(SCALE)
                                    )
                                    nc.tensor.matmul(
                                        dn[:], ones_sb[:], pt[:],
                                        start=(kt == 0), stop=(kt == 15),
                                    )
                                    nc.tensor.matmul(
                                        at[:],
                                        vp[:, kt, h2 * P : (h2 + 1) * P],
                                        pt[:],
                                        start=(kt == 0),
                                        stop=(kt == 15),
                                    )
                                rec = p3s.tile([P, TQ], F32, tag="rec")
                                nc.vector.reciprocal(rec[:], dn[:])
                                ast = p3s.tile([P, TQ], F32R, tag="ast")
                                nc.vector.tensor_tensor(
                                    ast[:], at[:], rec[:],
                                    mybir.AluOpType.mult,
                                )
                                nc.sync.dma_start(attnd[:, qh, :], ast[:])

            # ---------------- P4: o_proj ------------------------------------
            with (
                tc.tile_pool(name="p4s", bufs=2) as p4s,
                tc.tile_pool(name="p4ps", bufs=2, space="PSUM") as p4ps,
            ):
                for n in range(4):
                    ow = p4s.tile([P, 16, 512], F32R, tag="ow")
                    nc.sync.dma_start(
                        ow[:],
                        o_w.rearrange("(ht p) c -> p ht c", p=P)[
                            :, :, n * 512 : (n + 1) * 512
                        ],
                    )
                    for mt in range(4):
                        am = p4s.tile([P, NH, P], F32R, tag="am")
                        nc.sync.dma_start(am[:], attnd[:, :, mt * P : (mt + 1) * P])
                        ps = p4ps.tile([P, 512], F32, tag="o_ps")
                        for h in range(NH):
                            nc.tensor.matmul(
                                ps[:],
                                am[:, h, :],
                                ow[:, h, :],
                                start=(h == 0),
                                stop=(h == 15),
                            )
                        st = p4s.tile([P, 512], mybir.dt.float32, tag="ost")
                        nc.scalar.copy(st[:], ps[:])
                        nc.sync.dma_start(
                            out[mt * P : (mt + 1) * P, n * 512 : (n + 1) * 512], st[:]
                        )

    nc.finalize()
    return nc


def _host_prep(inputs):
    r = _round_f32r
    x = np.asarray(inputs["hidden_states"], dtype=np.float32)
    qa_w = r(inputs["q_a_w"])
    qa_ln = r(
        (np.asarray(inputs["q_a_ln_w"], np.float64) * math.sqrt(Q_RANK))
        .astype(np.float32)
        .reshape(Q_RANK // P, P)
        .T.copy()
    )
    kva_ln = r(
        (np.asarray(inputs["kv_a_ln_w"], np.float64) * math.sqrt(KV_RANK))
        .astype(np.float32)
        .reshape(KV_RANK // P, P)
        .T.copy()
    )
    o_w = r(inputs["o_w"])

    qb = np.asarray(inputs["q_b_w"], np.float32).reshape(Q_RANK, NH, HD)
    nope_cols = qb[:, :, :NOPE].reshape(Q_RANK, NH * NOPE)
    rope1 = qb[:, :, NOPE : NOPE + 32].reshape(Q_RANK, 16 * 32)
    rope2 = qb[:, :, NOPE + 32 :].reshape(Q_RANK, 16 * 32)
    qb_w = r(np.concatenate([nope_cols, rope1, rope2], axis=1))

    kva = np.asarray(inputs["kv_a_w"], np.float32)
    lat = kva[:, :KV_RANK]
    krope = kva[:, KV_RANK:].reshape(HID, NKV, ROPE)
    kr1 = krope[:, :, :32].reshape(HID, NKV * 32)
    kr2 = krope[:, :, 32:].reshape(HID, NKV * 32)
    kva_w = r(np.concatenate([lat, kr1, kr2], axis=1))

    kvb = np.asarray(inputs["kv_b_w"], np.float32).reshape(KV_RANK, NKV, NOPE + VD)
    knope_cols = kvb[:, :, :NOPE].reshape(KV_RANK, NKV * NOPE)
    v_cols = kvb[:, :, NOPE:].reshape(KV_RANK, NKV * VD)
    kvb_w = r(np.concatenate([knope_cols, v_cols], axis=1))

    inv_freq = 1.0 / (THETA ** (np.arange(0, ROPE, 2, dtype=np.float32) / ROPE))
    t = np.arange(T, dtype=np.float32)
    freqs = np.outer(t, inv_freq).astype(np.float32)
    cosk = r(np.tile(np.cos(freqs).T, (4, 1)))  # [128, T]
    sink = r(np.tile(np.sin(freqs).T, (4, 1)))
    ones = np.ones((P, P), np.float32)

    in_maps = []
    for c in range(NCORES):
        b, qc = c // 4, c % 4
        xTb = r(x[b].T.copy())
        qoff = qc * TQ
        in_maps.append(
            {
                "xT": xTb,
                "xq": np.ascontiguousarray(xTb[:, qoff : qoff + TQ]),
                "qa_w": qa_w,
                "qa_ln": qa_ln,
                "qb_w": qb_w,
                "kva_w": kva_w,
                "kva_ln": kva_ln,
                "kvb_w": kvb_w,
                "o_w": o_w,
                "cosq": np.ascontiguousarray(cosk[:, qoff : qoff + TQ]),
                "sinq": np.ascontiguousarray(sink[:, qoff : qoff + TQ]),
                "cosk": cosk,
                "sink": sink,
                "ones_in": ones,
            }
        )
    return in_maps


def get_nc():
    if "nc" not in _CACHE:
        _CACHE["nc"] = _build_nc()
    return _CACHE["nc"]


def kernel(**inputs) -> np.ndarray:
    from concourse.bass_utils import run_bass_kernel_spmd

    nc = get_nc()
    in_maps = _host_prep(inputs)
    res = run_bass_kernel_spmd(nc, in_maps, core_ids=list(range(NCORES)))
    _CACHE["last_result"] = res
    outs = [res.results[c]["out"] for c in range(NCORES)]
    full = np.stack(
        [np.concatenate([outs[b * 4 + qc] for qc in range(4)], axis=0) for b in range(B)]
    )
    return full.astype(np.float32)


# revision 10
# speedup vs baseline: 1.1543x; 1.1543x over previous
"""Multi-head latent attention (MLA) TRN2 kernel.

Sharding: batch(2) x query-sequence(4) over 8 cores. Each core:
  - computes the full KV path for its batch (kv_a, rmsnorm, kv_b, rope)
  - computes the Q path for its 512-token query chunk
  - full attention for its 512 queries x 2048 keys x 16 heads
  - o_proj for its chunk -> output slice [512, 2048]
Host assembles the 8 slices into [B, T, HID]. No collectives.

All matmuls run in float32r (fp32 with 11-bit mantissa, 1 cycle/row on the
PE when N>=256 -- same throughput as bf16 at ~2^-12 relative precision).
Activations are kept feature-major ([feature, token]) so weight matrices act
as lhsT directly as stored; attention computes scores transposed
(s^T[k,q] = k^T q) so softmax needs no transposes: exp on ACT, the
denominator via an all-ones lhsT matmul (broadcast into all 128 partitions),
and P@V consumes the transposed probabilities directly.
"""

import math

import numpy as np

B, T, HID = 2, 2048, 2048
NH, NKV = 16, 8
NOPE, ROPE = 128, 64
HD = NOPE + ROPE  # 192
VD = 128
KV_RANK, Q_RANK = 512, 1536
EPS = 1e-6
THETA = 10000.0
NCORES = 8
TQ = B * T // NCORES  # 512 query tokens per core
P = 128
SCALE = 1.0 / math.sqrt(HD)

# Rope rows are stored "paired": each head's rotated rope halves (32+32 rows)
# are stacked into one contiguous 64-row slot, two heads per 128-partition
# tile, at base partition 64*(kvh%2) so score-matmul lhsT(k)/rhs(q) base
# partitions match (PE only allows bases {0, 32, 64}).

_CACHE = {}


def _round_f32r(a):
    a = np.ascontiguousarray(np.asarray(a, dtype=np.float32))
    u = a.view(np.uint32)
    low = u & np.uint32(0xFFF)
    rounded = u & np.uint32(0xFFFFF000)
    lsb = (u >> np.uint32(12)) & np.uint32(1)
    round_up = (low > 0x800) | ((low == 0x800) & (lsb == 1))
    return (rounded + (round_up.astype(np.uint32) << np.uint32(12))).view(np.float32)


def _build_nc():
    import concourse.bass as bass  # noqa: F401
    import concourse.mybir as mybir
    from concourse import bacc
    from concourse.tile import TileContext

    F32 = mybir.dt.float32
    F32R = mybir.dt.float32r
    AF = mybir.ActivationFunctionType
    ALU = mybir.AluOpType

    nc = bacc.Bacc(None, target_bir_lowering=False)

    xT = nc.dram_tensor("xT", [HID, T], F32R, kind="ExternalInput")
    xq = nc.dram_tensor("xq", [HID, TQ], F32R, kind="ExternalInput")
    qa_w = nc.dram_tensor("qa_w", [HID, Q_RANK], F32R, kind="ExternalInput")
    qa_ln = nc.dram_tensor("qa_ln", [P, Q_RANK // P], F32R, kind="ExternalInput")
    qb_w = nc.dram_tensor("qb_w", [Q_RANK, NH * HD], F32R, kind="ExternalInput")
    kva_w = nc.dram_tensor("kva_w", [HID, KV_RANK + NKV * ROPE], F32R, kind="ExternalInput")
    kva_ln = nc.dram_tensor("kva_ln", [P, KV_RANK // P], F32R, kind="ExternalInput")
    kvb_w = nc.dram_tensor("kvb_w", [KV_RANK, NKV * (NOPE + VD)], F32R, kind="ExternalInput")
    o_w = nc.dram_tensor("o_w", [NH * VD, HID], F32R, kind="ExternalInput")
    cosq = nc.dram_tensor("cosq", [P, TQ], F32R, kind="ExternalInput")
    sinq = nc.dram_tensor("sinq", [P, TQ], F32R, kind="ExternalInput")
    cosk = nc.dram_tensor("cosk", [P, T], F32R, kind="ExternalInput")
    sink = nc.dram_tensor("sink", [P, T], F32R, kind="ExternalInput")
    ones_in = nc.dram_tensor("ones_in", [P, P], F32R, kind="ExternalInput")
    out = nc.dram_tensor("out", [TQ, HID], F32, kind="ExternalOutput")

    xT_t = xT.rearrange("(kt p) t -> p kt t", p=P)  # [128, 16, T]
    xq_t = xq.rearrange("(kt p) t -> p kt t", p=P)  # [128, 16, TQ]

    with TileContext(nc) as tc:
        with (
            tc.tile_pool(name="tables", bufs=1) as tbl,
            tc.tile_pool(name="dram", bufs=1, space="DRAM") as dpool,
            tc.tile_pool(name="pLat", bufs=1) as pLat,
        ):
            ones_sb = tbl.tile([P, P], F32R, name="ones_sb")
            nc.sync.dma_start(ones_sb[:], ones_in[:, :])
            lnq_sb = tbl.tile([P, Q_RANK // P], F32R, name="lnq_sb")
            nc.sync.dma_start(lnq_sb[:], qa_ln[:, :])
            lnkv_sb = tbl.tile([P, KV_RANK // P], F32R, name="lnkv_sb")
            nc.sync.dma_start(lnkv_sb[:], kva_ln[:, :])
            epskv_sb = tbl.tile([P, 1], F32, name="epskv_sb")
            nc.gpsimd.memset(epskv_sb[:], float(EPS * KV_RANK))
            epsq_sb = tbl.tile([P, 1], F32, name="epsq_sb")
            nc.gpsimd.memset(epsq_sb[:], float(EPS * Q_RANK))

            kpaird = dpool.tile([P, 4, T], F32R, name="kpaird")
            qnoped = dpool.tile([P, NH, TQ], F32R, name="qnoped")
            qpaird = dpool.tile([P, 8, TQ], F32R, name="qpaird")
            attnd = dpool.tile([P, NH, TQ], F32R, name="attnd")

            kv_latN = pLat.tile([P, 4, T], F32R, name="kv_latN")

            # ---------------- P2: q path (runs first; no kv deps) -----------
            with (
                tc.tile_pool(name="p2", bufs=1) as p2,
                tc.tile_pool(name="p2s", bufs=2) as p2s,
                tc.tile_pool(name="p2ps", bufs=2, space="PSUM") as p2ps,
                tc.tile_pool(name="p2ps1", bufs=1, space="PSUM") as p2ps1,
            ):
                q_lat = p2.tile([P, Q_RANK // P, TQ], F32R, name="q_lat")
                xq_lo = p2.tile([P, 8, TQ], F32R, name="xq_lo")
                nc.sync.dma_start(xq_lo[:], xq_t[:, 0:8, :])
                xq_hi = p2.tile([P, 8, TQ], F32R, name="xq_hi")
                nc.sync.dma_start(xq_hi[:], xq_t[:, 8:16, :])

                def xq_at(k):
                    return xq_lo[:, k, :] if k < 8 else xq_hi[:, k - 8, :]

                # q_a + rmsnorm
                sumsq = p2ps1.tile([P, TQ], F32, tag="qsumsq")
                for m in range(12):
                    wt = p2s.tile([P, 16, P], F32R, tag="qa_wt")
                    nc.sync.dma_start(
                        wt[:],
                        qa_w.rearrange("(kt p) c -> p kt c", p=P)[
                            :, :, m * P : (m + 1) * P
                        ],
                    )
                    ps = p2ps.tile([P, TQ], F32, tag="qa_ps")
                    for k in range(16):
                        nc.tensor.matmul(
                            ps[:], wt[:, k, :], xq_at(k),
                            start=(k == 0), stop=(k == 15),
                        )
                    nc.vector.tensor_copy(q_lat[:, m, :], ps[:])
                    sq = p2s.tile([P, TQ], F32R, tag="qsq")
                    nc.scalar.square(sq[:], ps[:])
                    nc.tensor.matmul(
                        sumsq[:], ones_sb[:], sq[:],
                        start=(m == 0), stop=(m == 11),
                    )
                sqt = p2s.tile([P, TQ], F32, tag="qsqt")
                nc.scalar.activation(sqt[:], sumsq[:], AF.Sqrt, bias=epsq_sb[:])
                rs_q = p2.tile([P, TQ], F32, name="rs_q")
                nc.vector.reciprocal(rs_q[:], sqt[:])
                for m in range(Q_RANK // P):
                    nc.vector.scalar_tensor_tensor(
                        q_lat[:, m, :], q_lat[:, m, :],
                        lnq_sb[:, m : m + 1], rs_q[:],
                        ALU.mult, ALU.mult,
                    )

                # q_b: nope tiles spill straight to HBM; rope raw kept for rot
                qraw1 = p2.tile([P, 4, TQ], F32R, name="qraw1")
                qraw2 = p2.tile([P, 4, TQ], F32R, name="qraw2")
                for m in range(24):
                    wt = p2s.tile([P, 12, P], F32R, tag="qb_wt")
                    nc.sync.dma_start(
                        wt[:],
                        qb_w.rearrange("(kt p) c -> p kt c", p=P)[
                            :, :, m * P : (m + 1) * P
                        ],
                    )
                    ps = p2ps.tile([P, TQ], F32, tag="qb_ps")
                    for k in range(12):
                        nc.tensor.matmul(
                            ps[:], wt[:, k, :], q_lat[:, k, :],
                            start=(k == 0), stop=(k == 11),
                        )
                    if m < 16:
                        st = p2s.tile([P, TQ], F32R, tag="qn_st")
                        nc.scalar.copy(st[:], ps[:])
                        nc.sync.dma_start(qnoped[:, m, :], st[:])
                    elif m < 20:
                        nc.scalar.copy(qraw1[:, m - 16, :], ps[:])
                    else:
                        nc.scalar.copy(qraw2[:, m - 20, :], ps[:])

                # q-rope rotation (batched) then scatter to paired HBM layout
                cosq_sb = p2.tile([P, TQ], F32R, name="cosq_sb")
                nc.sync.dma_start(cosq_sb[:], cosq[:, :])
                sinq_sb = p2.tile([P, TQ], F32R, name="sinq_sb")
                nc.sync.dma_start(sinq_sb[:], sinq[:, :])
                cb = cosq_sb[:, None, :].to_broadcast((P, 4, TQ))
                sb = sinq_sb[:, None, :].to_broadcast((P, 4, TQ))
                qrot1 = p2.tile([P, 4, TQ], F32R, name="qrot1")
                qrot2 = p2.tile([P, 4, TQ], F32R, name="qrot2")
                tmp = p2.tile([P, 4, TQ], F32R, name="qrot_tmp1")
                nc.vector.tensor_tensor(tmp[:], qraw2[:], sb, ALU.mult)
                nc.vector.tensor_tensor(qrot1[:], qraw1[:], cb, ALU.mult)
                nc.vector.tensor_tensor(qrot1[:], qrot1[:], tmp[:], ALU.subtract)
                tmp2 = p2.tile([P, 4, TQ], F32R, name="qrot_tmp2")
                nc.vector.tensor_tensor(tmp2[:], qraw1[:], sb, ALU.mult)
                nc.vector.tensor_tensor(qrot2[:], qraw2[:], cb, ALU.mult)
                nc.vector.tensor_tensor(qrot2[:], qrot2[:], tmp2[:], ALU.add)
                # head h -> tile 2*(h//4)+h%2, base 64*((h//2)%2)
                for h in range(NH):
                    tq_ = 2 * (h // 4) + h % 2
                    bb = 64 * ((h // 2) % 2)
                    nc.sync.dma_start(
                        qpaird[bb : bb + 32, tq_, :],
                        qrot1[(h % 4) * 32 : (h % 4) * 32 + 32, h // 4, :],
                    )
                    nc.sync.dma_start(
                        qpaird[bb + 32 : bb + 64, tq_, :],
                        qrot2[(h % 4) * 32 : (h % 4) * 32 + 32, h // 4, :],
                    )

            # ---------------- P1: kv_a (latent+rope) + rmsnorm + rotation ---
            with (
                tc.tile_pool(name="p1", bufs=1) as p1,
                tc.tile_pool(name="p1s", bufs=2) as p1s,
                tc.tile_pool(name="p1ps", bufs=2, space="PSUM") as p1ps,
                tc.tile_pool(name="p1ps1", bufs=1, space="PSUM") as p1ps1,
            ):
                kvaw_lo = p1.tile([P, 8, KV_RANK + NKV * ROPE], F32R, name="kvaw_lo")
                nc.sync.dma_start(
                    kvaw_lo[:], kva_w.rearrange("(kt p) c -> p kt c", p=P)[:, 0:8, :]
                )
                kvaw_hi = p1.tile([P, 8, KV_RANK + NKV * ROPE], F32R, name="kvaw_hi")
                nc.sync.dma_start(
                    kvaw_hi[:], kva_w.rearrange("(kt p) c -> p kt c", p=P)[:, 8:16, :]
                )

                def kvaw_at(k, csl):
                    return kvaw_lo[:, k, csl] if k < 8 else kvaw_hi[:, k - 8, csl]

                raw1 = p1.tile([P, 2, T], F32R, name="kraw1")
                raw2 = p1.tile([P, 2, T], F32R, name="kraw2")
                rs_kv = p1.tile([P, 8, 256], F32, name="rs_kv")

                NCH = 8
                CW = T // NCH  # 256
                for nch in range(NCH):
                    xch = p1s.tile([P, 16, CW], F32R, tag="xch")
                    nc.sync.dma_start(xch[:], xT_t[:, :, nch * CW : (nch + 1) * CW])
                    sumsq = p1ps1.tile([P, CW], F32, tag="sumsq")
                    for m in range(8):
                        csl = slice(m * P, (m + 1) * P)
                        ps = p1ps.tile([P, CW], F32, tag="kva_ps")
                        for k in range(16):
                            nc.tensor.matmul(
                                ps[:], kvaw_at(k, csl), xch[:, k, :],
                                start=(k == 0), stop=(k == 15),
                            )
                        if m < 4:
                            nc.vector.tensor_copy(
                                kv_latN[:, m, nch * CW : (nch + 1) * CW], ps[:]
                            )
                            sq = p1s.tile([P, CW], F32R, tag="sq")
                            nc.scalar.square(sq[:], ps[:])
                            nc.tensor.matmul(
                                sumsq[:], ones_sb[:], sq[:],
                                start=(m == 0), stop=(m == 3),
                            )
                        elif m < 6:
                            nc.scalar.copy(
                                raw1[:, m - 4, nch * CW : (nch + 1) * CW], ps[:]
                            )
                        else:
                            nc.scalar.copy(
                                raw2[:, m - 6, nch * CW : (nch + 1) * CW], ps[:]
                            )
                    sqt = p1s.tile([P, CW], F32, tag="sqt")
                    nc.scalar.activation(sqt[:], sumsq[:], AF.Sqrt, bias=epskv_sb[:])
                    nc.vector.reciprocal(rs_kv[:, nch, :], sqt[:])
                    for m in range(4):
                        nc.vector.scalar_tensor_tensor(
                            kv_latN[:, m, nch * CW : (nch + 1) * CW],
                            kv_latN[:, m, nch * CW : (nch + 1) * CW],
                            lnkv_sb[:, m : m + 1],
                            rs_kv[:, nch, :],
                            ALU.mult,
                            ALU.mult,
                        )

                # k-rope rotation (block layout, batched DVE) -> paired HBM
                cosk_sb = p1.tile([P, T], F32R, name="cosk_sb")
                nc.sync.dma_start(cosk_sb[:], cosk[:, :])
                sink_sb = p1.tile([P, T], F32R, name="sink_sb")
                nc.sync.dma_start(sink_sb[:], sink[:, :])
                HT = T // 2
                for t in range(2):
                    for hh in range(2):
                        tsl = slice(hh * HT, (hh + 1) * HT)
                        tmp = p1s.tile([P, HT], F32R, tag="rot_tmp")
                        rot = p1s.tile([P, HT], F32R, tag="rot_out")
                        nc.vector.tensor_tensor(
                            tmp[:], raw2[:, t, tsl], sink_sb[:, tsl], ALU.mult
                        )
                        nc.vector.tensor_tensor(
                            rot[:], raw1[:, t, tsl], cosk_sb[:, tsl], ALU.mult
                        )
                        nc.vector.tensor_tensor(rot[:], rot[:], tmp[:], ALU.subtract)
                        tmp2 = p1s.tile([P, HT], F32R, tag="rot_tmp")
                        rot2 = p1s.tile([P, HT], F32R, tag="rot_out")
                        nc.vector.tensor_tensor(
                            tmp2[:], raw1[:, t, tsl], sink_sb[:, tsl], ALU.mult
                        )
                        nc.vector.tensor_tensor(
                            rot2[:], raw2[:, t, tsl], cosk_sb[:, tsl], ALU.mult
                        )
                        nc.vector.tensor_tensor(rot2[:], rot2[:], tmp2[:], ALU.add)
                        # head kvh=4t+i -> tile kvh//2, base 64*(kvh%2)
                        for i in range(4):
                            kvh = 4 * t + i
                            bb = 64 * (kvh % 2)
                            nc.sync.dma_start(
                                kpaird[bb : bb + 32, kvh // 2, tsl],
                                rot[i * 32 : (i + 1) * 32, :],
                            )
                            nc.sync.dma_start(
                                kpaird[bb + 32 : bb + 64, kvh // 2, tsl],
                                rot2[i * 32 : (i + 1) * 32, :],
                            )

            # ---------------- P3: attention ---------------------------------
            with (
                tc.tile_pool(name="p3s", bufs=2) as p3s,
                tc.tile_pool(name="p3q", bufs=4) as p3q,
                tc.tile_pool(name="p3p", bufs=3) as p3p,
                tc.tile_pool(name="scps", bufs=3, space="PSUM") as scps,
                tc.tile_pool(name="dnps", bufs=1, space="PSUM") as dnps,
                tc.tile_pool(name="atps", bufs=2, space="PSUM") as atps,
                tc.tile_pool(name="prps", bufs=1, space="PSUM") as prps,
            ):
                for hp in range(4):  # kv-head pairs
                    kvh0 = 2 * hp
                    wn = p3s.tile([P, 4, 256], F32R, tag="wn")
                    nc.sync.dma_start(
                        wn[:],
                        kvb_w.rearrange("(kt p) c -> p kt c", p=P)[
                            :, :, kvh0 * NOPE : (kvh0 + 2) * NOPE
                        ],
                    )
                    wv = p3s.tile([P, 4, 256], F32R, tag="wv")
                    nc.sync.dma_start(
                        wv[:],
                        kvb_w.rearrange("(kt p) c -> p kt c", p=P)[
                            :, :, NKV * NOPE + kvh0 * VD : NKV * NOPE + (kvh0 + 2) * VD
                        ],
                    )
                    knp = p3s.tile([P, 2, T], F32R, tag="knp")
                    for h2 in range(2):
                        for nch in range(4):
                            ps = prps.tile([P, 512], F32, tag="knp_ps")
                            for k in range(4):
                                nc.tensor.matmul(
                                    ps[:],
                                    wn[:, k, h2 * P : (h2 + 1) * P],
                                    kv_latN[:, k, nch * 512 : (nch + 1) * 512],
                                    start=(k == 0),
                                    stop=(k == 3),
                                )
                            nc.vector.tensor_copy(
                                knp[:, h2, nch * 512 : (nch + 1) * 512], ps[:]
                            )
                    vp = p3s.tile([P, 16, 256], F32R, tag="vp")
                    for mt in range(16):
                        ps = prps.tile([P, 256], F32, tag="v_ps")
                        for k in range(4):
                            nc.tensor.matmul(
                                ps[:],
                                kv_latN[:, k, mt * P : (mt + 1) * P],
                                wv[:, k, :],
                                start=(k == 0),
                                stop=(k == 3),
                            )
                        nc.vector.tensor_copy(vp[:, mt, :], ps[:])
                    krp = p3s.tile([P, T], F32R, tag="krp")
                    nc.sync.dma_start(krp[:], kpaird[:, hp, :])
                    qps = {}
                    for tq_ in (2 * hp, 2 * hp + 1):
                        qp = p3q.tile([P, TQ], F32R, tag="qp")
                        nc.sync.dma_start(qp[:], qpaird[:, tq_, :])
                        qps[tq_] = qp

                    for j4 in range(4):
                        qh = 4 * hp + j4
                        kvh = qh // 2
                        h2 = kvh - kvh0
                        b = 64 * (kvh % 2)
                        tq_ = 2 * (qh // 4) + qh % 2
                        qn = p3q.tile([P, TQ], F32R, tag="qn")
                        nc.sync.dma_start(qn[:], qnoped[:, qh, :])
                        qp = qps[tq_]
                        dsum = p3q.tile([P, TQ], F32R, tag="dsum")
                        at = atps.tile([P, TQ], F32, tag="at")
                        for kt in range(16):
                            sc = scps.tile([P, TQ], F32, tag="sc")
                            nc.tensor.matmul(
                                sc[:],
                                knp[:, h2, kt * P : (kt + 1) * P],
                                qn[:],
                                start=True,
                                stop=False,
                            )
                            nc.tensor.matmul(
                                sc[:],
                                krp[b : b + 64, kt * P : (kt + 1) * P],
                                qp[b : b + 64, :],
                                start=False,
                                stop=True,
                            )
                            pt = p3p.tile([P, TQ], F32R, tag="probsT")
                            nc.scalar.activation(
                                pt[:], sc[:], AF.Exp, scale=float(SCALE)
                            )
                            if kt == 0:
                                nc.vector.tensor_copy(dsum[:], pt[:])
                            else:
                                nc.vector.tensor_tensor(
                                    dsum[:], dsum[:], pt[:], ALU.add
                                )
                            nc.tensor.matmul(
                                at[:],
                                vp[:, kt, h2 * P : (h2 + 1) * P],
                                pt[:],
                                start=(kt == 0),
                                stop=(kt == 15),
                            )
                        dn = dnps.tile([P, TQ], F32, tag="dn")
                        nc.tensor.matmul(
                            dn[:], ones_sb[:], dsum[:], start=True, stop=True
                        )
                        rec = p3q.tile([P, TQ], F32, tag="rec")
                        nc.vector.reciprocal(rec[:], dn[:])
                        ast = p3q.tile([P, TQ], F32R, tag="ast")
                        nc.vector.tensor_tensor(
                            ast[:], at[:], rec[:], ALU.mult
                        )
                        nc.sync.dma_start(attnd[:, qh, :], ast[:])

            # ---------------- P4: o_proj ------------------------------------
            with (
                tc.tile_pool(name="p4s", bufs=2) as p4s,
                tc.tile_pool(name="p4ps", bufs=2, space="PSUM") as p4ps,
            ):
                for n in range(4):
                    ow = p4s.tile([P, 16, 512], F32R, tag="ow")
                    nc.sync.dma_start(
                        ow[:],
                        o_w.rearrange("(ht p) c -> p ht c", p=P)[
                            :, :, n * 512 : (n + 1) * 512
                        ],
                    )
                    for mt in range(4):
                        am = p4s.tile([P, NH, P], F32R, tag="am")
                        nc.sync.dma_start(am[:], attnd[:, :, mt * P : (mt + 1) * P])
                        ps = p4ps.tile([P, 512], F32, tag="o_ps")
                        for h in range(NH):
                            nc.tensor.matmul(
                                ps[:], am[:, h, :], ow[:, h, :],
                                start=(h == 0), stop=(h == 15),
                            )
                        st = p4s.tile([P, 512], mybir.dt.float32, tag="ost")
                        nc.scalar.copy(st[:], ps[:])
                        nc.sync.dma_start(
                            out[mt * P : (mt + 1) * P, n * 512 : (n + 1) * 512], st[:]
                        )

    nc.finalize()
    return nc


def _host_prep(inputs):
    r = _round_f32r
    x = np.asarray(inputs["hidden_states"], dtype=np.float32)
    qa_w = r(inputs["q_a_w"])
    qa_ln = r(
        (np.asarray(inputs["q_a_ln_w"], np.float64) * math.sqrt(Q_RANK))
        .astype(np.float32)
        .reshape(Q_RANK // P, P)
        .T.copy()
    )
    kva_ln = r(
        (np.asarray(inputs["kv_a_ln_w"], np.float64) * math.sqrt(KV_RANK))
        .astype(np.float32)
        .reshape(KV_RANK // P, P)
        .T.copy()
    )
    o_w = r(inputs["o_w"])

    qb = np.asarray(inputs["q_b_w"], np.float32).reshape(Q_RANK, NH, HD)
    nope_cols = qb[:, :, :NOPE].reshape(Q_RANK, NH * NOPE)
    rope1 = qb[:, :, NOPE : NOPE + 32].reshape(Q_RANK, 16 * 32)
    rope2 = qb[:, :, NOPE + 32 :].reshape(Q_RANK, 16 * 32)
    qb_w = r(np.concatenate([nope_cols, rope1, rope2], axis=1))

    kva = np.asarray(inputs["kv_a_w"], np.float32)
    lat = kva[:, :KV_RANK]
    krope = kva[:, KV_RANK:].reshape(HID, NKV, ROPE)
    kr1 = krope[:, :, :32].reshape(HID, NKV * 32)
    kr2 = krope[:, :, 32:].reshape(HID, NKV * 32)
    kva_w = r(np.concatenate([lat, kr1, kr2], axis=1))

    kvb = np.asarray(inputs["kv_b_w"], np.float32).reshape(KV_RANK, NKV, NOPE + VD)
    knope_cols = kvb[:, :, :NOPE].reshape(KV_RANK, NKV * NOPE)
    v_cols = kvb[:, :, NOPE:].reshape(KV_RANK, NKV * VD)
    kvb_w = r(np.concatenate([knope_cols, v_cols], axis=1))

    inv_freq = 1.0 / (THETA ** (np.arange(0, ROPE, 2, dtype=np.float32) / ROPE))
    t = np.arange(T, dtype=np.float32)
    freqs = np.outer(t, inv_freq).astype(np.float32)
    cosk = r(np.tile(np.cos(freqs).T, (4, 1)))  # [128, T]
    sink = r(np.tile(np.sin(freqs).T, (4, 1)))
    ones = np.ones((P, P), np.float32)

    in_maps = []
    for c in range(NCORES):
        b, qc = c // 4, c % 4
        xTb = r(x[b].T.copy())
        qoff = qc * TQ
        in_maps.append(
            {
                "xT": xTb,
                "xq": np.ascontiguousarray(xTb[:, qoff : qoff + TQ]),
                "qa_w": qa_w,
                "qa_ln": qa_ln,
                "qb_w": qb_w,
                "kva_w": kva_w,
                "kva_ln": kva_ln,
                "kvb_w": kvb_w,
                "o_w": o_w,
                "cosq": np.ascontiguousarray(cosk[:, qoff : qoff + TQ]),
                "sinq": np.ascontiguousarray(sink[:, qoff : qoff + TQ]),
                "cosk": cosk,
                "sink": sink,
                "ones_in": ones,
            }
        )
    return in_maps


def get_nc():
    if "nc" not in _CACHE:
        _CACHE["nc"] = _build_nc()
    return _CACHE["nc"]


def kernel(**inputs) -> np.ndarray:
    from concourse.bass_utils import run_bass_kernel_spmd

    nc = get_nc()
    in_maps = _host_prep(inputs)
    res = run_bass_kernel_spmd(nc, in_maps, core_ids=list(range(NCORES)))
    _CACHE["last_result"] = res
    outs = [res.results[c]["out"] for c in range(NCORES)]
    full = np.stack(
        [np.concatenate([outs[b * 4 + qc] for qc in range(4)], axis=0) for b in range(B)]
    )
    return full.astype(np.float32)


# revision 18
# speedup vs baseline: 1.2676x; 1.0982x over previous
"""Multi-head latent attention (MLA) TRN2 kernel.

Sharding: batch(2) x query-sequence(4) over 8 cores. Each core:
  - computes the full KV path for its batch (kv_a, rmsnorm, kv_b, rope)
  - computes the Q path for its 512-token query chunk
  - full attention for its 512 queries x 2048 keys x 16 heads
  - o_proj for its chunk -> output slice [512, 2048]
Host assembles the 8 slices into [B, T, HID]. No collectives.

All matmuls run in float32r (fp32 with 11-bit mantissa, 1 cycle/row on the
PE when N>=256 -- same throughput as bf16 at ~2^-12 relative precision).
Activations are kept feature-major ([feature, token]) so weight matrices act
as lhsT directly as stored; attention computes scores transposed
(s^T[k,q] = k^T q) so softmax needs no transposes: exp on ACT, the
denominator via an all-ones lhsT matmul (broadcast into all 128 partitions),
and P@V consumes the transposed probabilities directly.
"""

import math

import numpy as np

B, T, HID = 2, 2048, 2048
NH, NKV = 16, 8
NOPE, ROPE = 128, 64
HD = NOPE + ROPE  # 192
VD = 128
KV_RANK, Q_RANK = 512, 1536
EPS = 1e-6
THETA = 10000.0
NCORES = 8
TQ = B * T // NCORES  # 512 query tokens per core
P = 128
SCALE = 1.0 / math.sqrt(HD)

# Rope rows are stored "paired": each head's rotated rope halves (32+32 rows)
# are stacked into one contiguous 64-row slot, two heads per 128-partition
# tile, at base partition 64*(kvh%2) so score-matmul lhsT(k)/rhs(q) base
# partitions match (PE only allows bases {0, 32, 64}).

_CACHE = {}


def _round_f32r(a):
    a = np.ascontiguousarray(np.asarray(a, dtype=np.float32))
    u = a.view(np.uint32)
    low = u & np.uint32(0xFFF)
    rounded = u & np.uint32(0xFFFFF000)
    lsb = (u >> np.uint32(12)) & np.uint32(1)
    round_up = (low > 0x800) | ((low == 0x800) & (lsb == 1))
    return (rounded + (round_up.astype(np.uint32) << np.uint32(12))).view(np.float32)


def _build_nc():
    import concourse.bass as bass  # noqa: F401
    import concourse.mybir as mybir
    from concourse import bacc
    from concourse.tile import TileContext

    F32 = mybir.dt.float32
    F32R = mybir.dt.float32r
    AF = mybir.ActivationFunctionType
    ALU = mybir.AluOpType

    nc = bacc.Bacc(None, target_bir_lowering=False)

    xT = nc.dram_tensor("xT", [HID, T], F32R, kind="ExternalInput")
    xq = nc.dram_tensor("xq", [HID, TQ], F32R, kind="ExternalInput")
    qa_w = nc.dram_tensor("qa_w", [HID, Q_RANK], F32R, kind="ExternalInput")
    qa_ln = nc.dram_tensor("qa_ln", [P, Q_RANK // P], F32R, kind="ExternalInput")
    qb_w = nc.dram_tensor("qb_w", [Q_RANK, NH * HD], F32R, kind="ExternalInput")
    kva_w = nc.dram_tensor("kva_w", [HID, KV_RANK + NKV * ROPE], F32R, kind="ExternalInput")
    kva_ln = nc.dram_tensor("kva_ln", [P, KV_RANK // P], F32R, kind="ExternalInput")
    kvb_w = nc.dram_tensor("kvb_w", [KV_RANK, NKV * (NOPE + VD)], F32R, kind="ExternalInput")
    o_w = nc.dram_tensor("o_w", [NH * VD, HID], F32R, kind="ExternalInput")
    cosq = nc.dram_tensor("cosq", [P, TQ], F32R, kind="ExternalInput")
    sinq = nc.dram_tensor("sinq", [P, TQ], F32R, kind="ExternalInput")
    cosk = nc.dram_tensor("cosk", [P, T], F32R, kind="ExternalInput")
    sink = nc.dram_tensor("sink", [P, T], F32R, kind="ExternalInput")
    ones_in = nc.dram_tensor("ones_in", [P, P], F32R, kind="ExternalInput")
    out = nc.dram_tensor("out", [TQ, HID], F32, kind="ExternalOutput")

    xT_t = xT.rearrange("(kt p) t -> p kt t", p=P)  # [128, 16, T]
    xq_t = xq.rearrange("(kt p) t -> p kt t", p=P)  # [128, 16, TQ]

    with TileContext(nc) as tc:
        with (
            tc.tile_pool(name="tables", bufs=1) as tbl,
            tc.tile_pool(name="dram", bufs=1, space="DRAM") as dpool,
            tc.tile_pool(name="pAttn", bufs=1) as pAttn,
        ):
            ones_sb = tbl.tile([P, P], F32R, name="ones_sb")
            nc.sync.dma_start(ones_sb[:], ones_in[:, :])
            lnq_sb = tbl.tile([P, Q_RANK // P], F32R, name="lnq_sb")
            nc.sync.dma_start(lnq_sb[:], qa_ln[:, :])
            lnkv_sb = tbl.tile([P, KV_RANK // P], F32R, name="lnkv_sb")
            nc.sync.dma_start(lnkv_sb[:], kva_ln[:, :])
            epskv_sb = tbl.tile([P, 1], F32, name="epskv_sb")
            nc.gpsimd.memset(epskv_sb[:], float(EPS * KV_RANK))
            epsq_sb = tbl.tile([P, 1], F32, name="epsq_sb")
            nc.gpsimd.memset(epsq_sb[:], float(EPS * Q_RANK))

            kpaird = dpool.tile([P, 4, T], F32R, name="kpaird")
            qnoped = dpool.tile([P, NH, TQ], F32R, name="qnoped")
            qpaird = dpool.tile([P, 8, TQ], F32R, name="qpaird")

            # attention output, resident through P3+P4
            attn_sb = pAttn.tile([P, NH, TQ], F32R, name="attn_sb")

            with tc.tile_pool(name="pLat", bufs=1) as pLat:
                kv_latN = pLat.tile([P, 4, T], F32R, name="kv_latN")

                # ------------- P2: q path (first; no kv deps) ---------------
                with (
                    tc.tile_pool(name="p2", bufs=1) as p2,
                    tc.tile_pool(name="p2s", bufs=2) as p2s,
                    tc.tile_pool(name="p2w", bufs=3) as p2w,
                    tc.tile_pool(name="p2ps", bufs=2, space="PSUM") as p2ps,
                    tc.tile_pool(name="p2ps1", bufs=1, space="PSUM") as p2ps1,
                ):
                    q_lat = p2.tile([P, Q_RANK // P, TQ], F32R, name="q_lat")
                    rs_q = p2.tile([P, TQ], F32, name="rs_q")

                    with tc.tile_pool(name="p2xq", bufs=1) as p2xq:
                        xq_c = []
                        for c in range(4):
                            t_ = p2xq.tile([P, 4, TQ], F32R, name=f"xq_c{c}")
                            nc.sync.dma_start(t_[:], xq_t[:, 4 * c : 4 * c + 4, :])
                            xq_c.append(t_)

                        # q_a + rmsnorm
                        sumsq = p2ps1.tile([P, TQ], F32, tag="qsumsq")
                        for m in range(12):
                            wt = p2w.tile([P, 16, P], F32R, tag="qa_wt")
                            nc.sync.dma_start(
                                wt[:],
                                qa_w.rearrange("(kt p) c -> p kt c", p=P)[
                                    :, :, m * P : (m + 1) * P
                                ],
                            )
                            ps = p2ps.tile([P, TQ], F32, tag="qa_ps")
                            for k in range(16):
                                nc.tensor.matmul(
                                    ps[:], wt[:, k, :], xq_c[k // 4][:, k % 4, :],
                                    start=(k == 0), stop=(k == 15),
                                )
                            nc.vector.tensor_copy(q_lat[:, m, :], ps[:])
                            sq = p2s.tile([P, TQ], F32R, tag="qsq")
                            nc.scalar.square(sq[:], ps[:])
                            nc.tensor.matmul(
                                sumsq[:], ones_sb[:], sq[:],
                                start=(m == 0), stop=(m == 11),
                            )
                        sqt = p2s.tile([P, TQ], F32, tag="qsqt")
                        nc.scalar.activation(sqt[:], sumsq[:], AF.Sqrt, bias=epsq_sb[:])
                        nc.vector.reciprocal(rs_q[:], sqt[:])
                        for m in range(Q_RANK // P):
                            nc.vector.scalar_tensor_tensor(
                                q_lat[:, m, :], q_lat[:, m, :],
                                lnq_sb[:, m : m + 1], rs_q[:],
                                ALU.mult, ALU.mult,
                            )

                    # q_b: nope tiles spill to HBM; rope raw kept for rotation
                    with tc.tile_pool(name="p2b", bufs=1) as p2b:
                        qraw1 = p2b.tile([P, 4, TQ], F32R, name="qraw1")
                        qraw2 = p2b.tile([P, 4, TQ], F32R, name="qraw2")
                        for m in range(24):
                            wt = p2w.tile([P, 12, P], F32R, tag="qb_wt")
                            nc.sync.dma_start(
                                wt[:],
                                qb_w.rearrange("(kt p) c -> p kt c", p=P)[
                                    :, :, m * P : (m + 1) * P
                                ],
                            )
                            ps = p2ps.tile([P, TQ], F32, tag="qb_ps")
                            for k in range(12):
                                nc.tensor.matmul(
                                    ps[:], wt[:, k, :], q_lat[:, k, :],
                                    start=(k == 0), stop=(k == 11),
                                )
                            if m < 16:
                                st = p2s.tile([P, TQ], F32R, tag="qn_st")
                                nc.scalar.copy(st[:], ps[:])
                                nc.sync.dma_start(qnoped[:, m, :], st[:])
                            elif m < 20:
                                nc.scalar.copy(qraw1[:, m - 16, :], ps[:])
                            else:
                                nc.scalar.copy(qraw2[:, m - 20, :], ps[:])

                        # q-rope rotation then scatter to paired HBM layout
                        cosq_sb = p2b.tile([P, TQ], F32R, name="cosq_sb")
                        nc.sync.dma_start(cosq_sb[:], cosq[:, :])
                        sinq_sb = p2b.tile([P, TQ], F32R, name="sinq_sb")
                        nc.sync.dma_start(sinq_sb[:], sinq[:, :])
                        cb = cosq_sb[:, None, :].to_broadcast((P, 4, TQ))
                        sb = sinq_sb[:, None, :].to_broadcast((P, 4, TQ))
                        qrot1 = p2b.tile([P, 4, TQ], F32R, name="qrot1")
                        qrot2 = p2b.tile([P, 4, TQ], F32R, name="qrot2")
                        tmp = p2b.tile([P, 4, TQ], F32R, name="qrot_tmp1")
                        nc.vector.tensor_tensor(tmp[:], qraw2[:], sb, ALU.mult)
                        nc.vector.tensor_tensor(qrot1[:], qraw1[:], cb, ALU.mult)
                        nc.vector.tensor_tensor(qrot1[:], qrot1[:], tmp[:], ALU.subtract)
                        tmp2 = p2b.tile([P, 4, TQ], F32R, name="qrot_tmp2")
                        nc.vector.tensor_tensor(tmp2[:], qraw1[:], sb, ALU.mult)
                        nc.vector.tensor_tensor(qrot2[:], qraw2[:], cb, ALU.mult)
                        nc.vector.tensor_tensor(qrot2[:], qrot2[:], tmp2[:], ALU.add)
                        # head h -> tile 2*(h//4)+h%2, base 64*((h//2)%2)
                        for h in range(NH):
                            tq_ = 2 * (h // 4) + h % 2
                            bb = 64 * ((h // 2) % 2)
                            nc.sync.dma_start(
                                qpaird[bb : bb + 32, tq_, :],
                                qrot1[(h % 4) * 32 : (h % 4) * 32 + 32, h // 4, :],
                            )
                            nc.sync.dma_start(
                                qpaird[bb + 32 : bb + 64, tq_, :],
                                qrot2[(h % 4) * 32 : (h % 4) * 32 + 32, h // 4, :],
                            )

                # ------------- P1: kv_a + rmsnorm + interleaved rotation ----
                with (
                    tc.tile_pool(name="p1", bufs=1) as p1,
                    tc.tile_pool(name="p1s", bufs=2) as p1s,
                    tc.tile_pool(name="p1ps", bufs=2, space="PSUM") as p1ps,
                    tc.tile_pool(name="p1ps1", bufs=1, space="PSUM") as p1ps1,
                ):
                    kvaw_c = []
                    for c in range(4):
                        t_ = p1.tile([P, 16, 256], F32R, name=f"kvaw_c{c}")
                        nc.sync.dma_start(
                            t_[:],
                            kva_w.rearrange("(kt p) c -> p kt c", p=P)[
                                :, :, c * 256 : (c + 1) * 256
                            ],
                        )
                        kvaw_c.append(t_)

                    def kvaw_at(k, m):
                        return kvaw_c[m // 2][:, k, (m % 2) * P : (m % 2 + 1) * P]

                    cosk_sb = p1.tile([P, T], F32R, name="cosk_sb")
                    nc.sync.dma_start(cosk_sb[:], cosk[:, :])
                    sink_sb = p1.tile([P, T], F32R, name="sink_sb")
                    nc.sync.dma_start(sink_sb[:], sink[:, :])
                    rs_kv = p1.tile([P, 8, 256], F32, name="rs_kv")

                    NCH = 8
                    CW = T // NCH  # 256
                    for nch in range(NCH):
                        chsl = slice(nch * CW, (nch + 1) * CW)
                        xch = p1s.tile([P, 16, CW], F32R, tag="xch")
                        nc.sync.dma_start(xch[:], xT_t[:, :, chsl])
                        sumsq = p1ps1.tile([P, CW], F32, tag="sumsq")
                        raw1 = p1s.tile([P, 2, CW], F32R, tag="kraw1")
                        raw2 = p1s.tile([P, 2, CW], F32R, tag="kraw2")
                        for m in range(8):
                            ps = p1ps.tile([P, CW], F32, tag="kva_ps")
                            for k in range(16):
                                nc.tensor.matmul(
                                    ps[:], kvaw_at(k, m), xch[:, k, :],
                                    start=(k == 0), stop=(k == 15),
                                )
                            if m < 4:
                                nc.vector.tensor_copy(kv_latN[:, m, chsl], ps[:])
                                sq = p1s.tile([P, CW], F32R, tag="sq")
                                nc.scalar.square(sq[:], ps[:])
                                nc.tensor.matmul(
                                    sumsq[:], ones_sb[:], sq[:],
                                    start=(m == 0), stop=(m == 3),
                                )
                            elif m < 6:
                                nc.scalar.copy(raw1[:, m - 4, :], ps[:])
                            else:
                                nc.scalar.copy(raw2[:, m - 6, :], ps[:])
                        sqt = p1s.tile([P, CW], F32, tag="sqt")
                        nc.scalar.activation(sqt[:], sumsq[:], AF.Sqrt, bias=epskv_sb[:])
                        nc.vector.reciprocal(rs_kv[:, nch, :], sqt[:])
                        for m in range(4):
                            nc.vector.scalar_tensor_tensor(
                                kv_latN[:, m, chsl],
                                kv_latN[:, m, chsl],
                                lnkv_sb[:, m : m + 1],
                                rs_kv[:, nch, :],
                                ALU.mult,
                                ALU.mult,
                            )
                        # rotate this chunk's rope rows and scatter to HBM
                        for t in range(2):
                            tmp = p1s.tile([P, CW], F32R, tag="rot_tmp")
                            rot = p1s.tile([P, CW], F32R, tag="rot_out")
                            nc.vector.tensor_tensor(
                                tmp[:], raw2[:, t, :], sink_sb[:, chsl], ALU.mult
                            )
                            nc.vector.tensor_tensor(
                                rot[:], raw1[:, t, :], cosk_sb[:, chsl], ALU.mult
                            )
                            nc.vector.tensor_tensor(rot[:], rot[:], tmp[:], ALU.subtract)
                            tmp2 = p1s.tile([P, CW], F32R, tag="rot_tmp")
                            rot2 = p1s.tile([P, CW], F32R, tag="rot_out")
                            nc.vector.tensor_tensor(
                                tmp2[:], raw1[:, t, :], sink_sb[:, chsl], ALU.mult
                            )
                            nc.vector.tensor_tensor(
                                rot2[:], raw2[:, t, :], cosk_sb[:, chsl], ALU.mult
                            )
                            nc.vector.tensor_tensor(rot2[:], rot2[:], tmp2[:], ALU.add)
                            # head kvh=4t+i -> tile kvh//2, base 64*(kvh%2)
                            for i in range(4):
                                kvh = 4 * t + i
                                bb = 64 * (kvh % 2)
                                nc.sync.dma_start(
                                    kpaird[bb : bb + 32, kvh // 2, chsl],
                                    rot[i * 32 : (i + 1) * 32, :],
                                )
                                nc.sync.dma_start(
                                    kpaird[bb + 32 : bb + 64, kvh // 2, chsl],
                                    rot2[i * 32 : (i + 1) * 32, :],
                                )

                # ------------- P3: attention --------------------------------
                with (
                    tc.tile_pool(name="p3s", bufs=2) as p3s,
                    tc.tile_pool(name="p3q", bufs=4) as p3q,
                    tc.tile_pool(name="p3p", bufs=3) as p3p,
                    tc.tile_pool(name="scps", bufs=3, space="PSUM") as scps,
                    tc.tile_pool(name="atps", bufs=2, space="PSUM") as atps,
                    tc.tile_pool(name="prps", bufs=2, space="PSUM") as prps,
                ):
                    pending = []

                    def finalize(item):
                        dsum, at, qh = item
                        dn = scps.tile([P, TQ], F32, tag="sc")
                        nc.tensor.matmul(
                            dn[:], ones_sb[:], dsum[:], start=True, stop=True
                        )
                        rec = p3q.tile([P, TQ], F32, tag="rec")
                        nc.vector.reciprocal(rec[:], dn[:])
                        nc.vector.tensor_tensor(
                            attn_sb[:, qh, :], at[:], rec[:], ALU.mult
                        )

                    for hp in range(4):  # kv-head pairs
                        kvh0 = 2 * hp
                        wn = p3s.tile([P, 4, 256], F32R, tag="wn")
                        nc.sync.dma_start(
                            wn[:],
                            kvb_w.rearrange("(kt p) c -> p kt c", p=P)[
                                :, :, kvh0 * NOPE : (kvh0 + 2) * NOPE
                            ],
                        )
                        wv = p3s.tile([P, 4, 256], F32R, tag="wv")
                        nc.sync.dma_start(
                            wv[:],
                            kvb_w.rearrange("(kt p) c -> p kt c", p=P)[
                                :, :, NKV * NOPE + kvh0 * VD : NKV * NOPE + (kvh0 + 2) * VD
                            ],
                        )
                        knp = p3s.tile([P, 2, T], F32R, tag="knp")
                        for h2 in range(2):
                            for nch in range(4):
                                ps = prps.tile([P, 512], F32, tag="pr_ps")
                                for k in range(4):
                                    nc.tensor.matmul(
                                        ps[:],
                                        wn[:, k, h2 * P : (h2 + 1) * P],
                                        kv_latN[:, k, nch * 512 : (nch + 1) * 512],
                                        start=(k == 0),
                                        stop=(k == 3),
                                    )
                                nc.vector.tensor_copy(
                                    knp[:, h2, nch * 512 : (nch + 1) * 512], ps[:]
                                )
                        vp = p3s.tile([P, 16, 256], F32R, tag="vp")
                        for mt in range(16):
                            psf = prps.tile([P, 512], F32, tag="pr_ps")
                            ps = psf[:, :256]
                            for k in range(4):
                                nc.tensor.matmul(
                                    ps[:],
                                    kv_latN[:, k, mt * P : (mt + 1) * P],
                                    wv[:, k, :],
                                    start=(k == 0),
                                    stop=(k == 3),
                                )
                            nc.vector.tensor_copy(vp[:, mt, :], ps[:])
                        krp = p3s.tile([P, T], F32R, tag="krp")
                        nc.sync.dma_start(krp[:], kpaird[:, hp, :])
                        qps = {}
                        for tq_ in (2 * hp, 2 * hp + 1):
                            qp = p3q.tile([P, TQ], F32R, tag="qp")
                            nc.sync.dma_start(qp[:], qpaird[:, tq_, :])
                            qps[tq_] = qp

                        for j4 in range(4):
                            qh = 4 * hp + j4
                            kvh = qh // 2
                            h2 = kvh - kvh0
                            b = 64 * (kvh % 2)
                            tq_ = 2 * (qh // 4) + qh % 2
                            qn = p3q.tile([P, TQ], F32R, tag="qn")
                            nc.sync.dma_start(qn[:], qnoped[:, qh, :])
                            qp = qps[tq_]
                            dsum = p3q.tile([P, TQ], F32R, tag="dsum")
                            at = atps.tile([P, TQ], F32, tag="at")
                            pts = {}
                            for kt in range(16):
                                sc = scps.tile([P, TQ], F32, tag="sc")
                                nc.tensor.matmul(
                                    sc[:],
                                    knp[:, h2, kt * P : (kt + 1) * P],
                                    qn[:],
                                    start=True,
                                    stop=False,
                                )
                                nc.tensor.matmul(
                                    sc[:],
                                    krp[b : b + 64, kt * P : (kt + 1) * P],
                                    qp[b : b + 64, :],
                                    start=False,
                                    stop=True,
                                )
                                pt = p3p.tile([P, TQ], F32R, tag="probsT")
                                nc.scalar.activation(
                                    pt[:], sc[:], AF.Exp, scale=float(SCALE)
                                )
                                pts[kt] = pt
                                if kt == 0:
                                    nc.vector.tensor_copy(dsum[:], pt[:])
                                else:
                                    nc.vector.tensor_tensor(
                                        dsum[:], dsum[:], pt[:], ALU.add
                                    )
                                if kt > 0:  # PV one stage behind scores
                                    nc.tensor.matmul(
                                        at[:],
                                        vp[:, kt - 1, h2 * P : (h2 + 1) * P],
                                        pts[kt - 1][:],
                                        start=(kt == 1),
                                        stop=False,
                                    )
                                    del pts[kt - 1]
                            nc.tensor.matmul(
                                at[:],
                                vp[:, 15, h2 * P : (h2 + 1) * P],
                                pts[15][:],
                                start=False,
                                stop=True,
                            )
                            pending.append((dsum, at, qh))
                            if len(pending) == 2:
                                finalize(pending.pop(0))
                    while pending:
                        finalize(pending.pop(0))

            # ------------- P4: o_proj (attn_sb resident) --------------------
            with (
                tc.tile_pool(name="p4s", bufs=2) as p4s,
                tc.tile_pool(name="p4ps", bufs=2, space="PSUM") as p4ps,
            ):
                for n in range(4):
                    ow = p4s.tile([P, 16, 512], F32R, tag="ow")
                    nc.sync.dma_start(
                        ow[:],
                        o_w.rearrange("(ht p) c -> p ht c", p=P)[
                            :, :, n * 512 : (n + 1) * 512
                        ],
                    )
                    for mt in range(4):
                        ps = p4ps.tile([P, 512], F32, tag="o_ps")
                        for h in range(NH):
                            nc.tensor.matmul(
                                ps[:],
                                attn_sb[:, h, mt * P : (mt + 1) * P],
                                ow[:, h, :],
                                start=(h == 0),
                                stop=(h == 15),
                            )
                        st = p4s.tile([P, 512], mybir.dt.float32, tag="ost")
                        nc.scalar.copy(st[:], ps[:])
                        nc.sync.dma_start(
                            out[mt * P : (mt + 1) * P, n * 512 : (n + 1) * 512], st[:]
                        )

    nc.finalize()
    return nc


def _host_prep(inputs):
    r = _round_f32r
    x = np.asarray(inputs["hidden_states"], dtype=np.float32)
    qa_w = r(inputs["q_a_w"])
    qa_ln = r(
        (np.asarray(inputs["q_a_ln_w"], np.float64) * math.sqrt(Q_RANK))
        .astype(np.float32)
        .reshape(Q_RANK // P, P)
        .T.copy()
    )
    kva_ln = r(
        (np.asarray(inputs["kv_a_ln_w"], np.float64) * math.sqrt(KV_RANK))
        .astype(np.float32)
        .reshape(KV_RANK // P, P)
        .T.copy()
    )
    o_w = r(inputs["o_w"])

    qb = np.asarray(inputs["q_b_w"], np.float32).reshape(Q_RANK, NH, HD)
    nope_cols = qb[:, :, :NOPE].reshape(Q_RANK, NH * NOPE)
    rope1 = qb[:, :, NOPE : NOPE + 32].reshape(Q_RANK, 16 * 32)
    rope2 = qb[:, :, NOPE + 32 :].reshape(Q_RANK, 16 * 32)
    qb_w = r(np.concatenate([nope_cols, rope1, rope2], axis=1))

    kva = np.asarray(inputs["kv_a_w"], np.float32)
    lat = kva[:, :KV_RANK]
    krope = kva[:, KV_RANK:].reshape(HID, NKV, ROPE)
    kr1 = krope[:, :, :32].reshape(HID, NKV * 32)
    kr2 = krope[:, :, 32:].reshape(HID, NKV * 32)
    kva_w = r(np.concatenate([lat, kr1, kr2], axis=1))

    kvb = np.asarray(inputs["kv_b_w"], np.float32).reshape(KV_RANK, NKV, NOPE + VD)
    knope_cols = kvb[:, :, :NOPE].reshape(KV_RANK, NKV * NOPE)
    v_cols = kvb[:, :, NOPE:].reshape(KV_RANK, NKV * VD)
    kvb_w = r(np.concatenate([knope_cols, v_cols], axis=1))

    inv_freq = 1.0 / (THETA ** (np.arange(0, ROPE, 2, dtype=np.float32) / ROPE))
    t = np.arange(T, dtype=np.float32)
    freqs = np.outer(t, inv_freq).astype(np.float32)
    cosk = r(np.tile(np.cos(freqs).T, (4, 1)))  # [128, T]
    sink = r(np.tile(np.sin(freqs).T, (4, 1)))
    ones = np.ones((P, P), np.float32)

    in_maps = []
    for c in range(NCORES):
        b, qc = c // 4, c % 4
        xTb = r(x[b].T.copy())
        qoff = qc * TQ
        in_maps.append(
            {
                "xT": xTb,
                "xq": np.ascontiguousarray(xTb[:, qoff : qoff + TQ]),
                "qa_w": qa_w,
                "qa_ln": qa_ln,
                "qb_w": qb_w,
                "kva_w": kva_w,
                "kva_ln": kva_ln,
                "kvb_w": kvb_w,
                "o_w": o_w,
                "cosq": np.ascontiguousarray(cosk[:, qoff : qoff + TQ]),
                "sinq": np.ascontiguousarray(sink[:, qoff : qoff + TQ]),
                "cosk": cosk,
                "sink": sink,
                "ones_in": ones,
            }
        )
    return in_maps


def get_nc():
    if "nc" not in _CACHE:
        _CACHE["nc"] = _build_nc()
    return _CACHE["nc"]


def kernel(**inputs) -> np.ndarray:
    from concourse.bass_utils import run_bass_kernel_spmd

    nc = get_nc()
    in_maps = _host_prep(inputs)
    res = run_bass_kernel_spmd(nc, in_maps, core_ids=list(range(NCORES)))
    _CACHE["last_result"] = res
    outs = [res.results[c]["out"] for c in range(NCORES)]
    full = np.stack(
        [np.concatenate([outs[b * 4 + qc] for qc in range(4)], axis=0) for b in range(B)]
    )
    return full.astype(np.float32)
